# revision 103
# baseline (speedup 1.0000x reference)
"""Local (windowed causal) attention Trainium2 kernel.

Problem: B=4, L=4096, D=1024, H=16 heads, dh=64, window W=128, causal
within each window. y = OutProj(Attn(QKV(x))).

Sharding: tokens are flattened to [16384, 1024] and split across 8
cores (2048 tokens = 16 complete windows per core). Fully data
parallel; weights are broadcast. No cross-core communication.

Per-core dataflow (v7 — xbar-transpose attention, residual fp8
DoubleRow projections):
  * x arrives pre-transposed from host as xT [1024, 2048], split into
    fp8-e4m3 high + residual parts (xh + xl ~= x); weights scaled by
    32 and split likewise. QK projections run 2 DR passes
    (wh*xh + wh*xl; exp() attenuation makes the dropped wl*xh term
    harmless), V and out-proj run 3 passes.
  * Attention per head-pair: scores on PE (the fully-masked quarter is
    pre-set to -1e30 in PSUM a call ahead), exp on ACT, causal mask as
    one 4x-mode DVE multiply by a constant mask tile, then P goes
    UNNORMALIZED through a single xbar-DMA transpose (DmaTransposeAnt,
    14ns/tile on the DMA engines — zero PE cost). attn@V uses the
    [q, dh] 64-column form (half the PE cost); softmax normalization
    happens on the attn@V output where per-q scaling is per-partition.
    aoT for the out-proj comes from a second xbar transpose; its fp8
    high+residual split runs on Pool.
  * Out-projection is TRANSPOSED (stationary = weights): y^T comes out
    [feat, token], the bias is per-partition (folds into the ACT
    epilogue), and the host un-transposes y. The last head-pair of
    every block (plus hp6 of wb3) runs the v5 inline path (PE
    transposes) so no attention chain ever crosses a block seam.
  * Scheduling: per-call in-order queues are kept shallow — attn@V
    runs two calls after its scores; xbar DMAs and y stores are
    EMITTED one call after their producers so the in-order SP
    sequencer never parks head-of-line; score PSUM banks rotate
    through 3 banks with their mask memsets issued a call ahead;
    epilogues alternate ACT/DVE. outp(wb) spreads over calls 3..7 of
    block wb+1. Dummy-transpose padding holds the PE p-state through
    the drained tail (any idle gap drops the PE clock to 0.65GHz).
  * DMA: HWDGE (SP) carries x + V-weights + P^T/aoT transposes + y
    stores; SWDGE carries w3/wo/biases sliced just ahead of use.
"""

import numpy as np

import concourse.bass as bass
import concourse.mybir as mybir
import concourse.tile as tile
from concourse.bass_utils import run_bass_kernel_spmd
from concourse.vector_clock import ScopedClock, VectorClock

# ---------------------------------------------------------------------------
# Workaround: the pinned walrus rejects any sync-wait on an SP-engine CTRL
# (drain) instruction ("Too many sync wait commands"). Emit the end-of-kernel
# global-clock waits on non-SP engine drains instead, one wait per drain.
# ---------------------------------------------------------------------------


def _drain_and_barrier_split(self, tick_clock, wait_clock):
    g = tick_clock.global_clock
    engines = [self.nc.scalar, self.nc.vector, self.nc.gpsimd, self.nc.tensor]
    for p, t in enumerate(list(g)):
        if t == 0:
            continue
        part = VectorClock()
        part.require_at_least(p, t)
        d = engines[p % len(engines)].drain()
        wait_clock.add_sem_waits(d.ins, ScopedClock({None: part}))
    self.nc.sync.drain()
    self.nc.all_engine_barrier()
    assert self.sems is not None
    popped = self.nc._tile_sem_poison_stack.pop()
    assert popped is self._sem_poison
    self.nc.clear_and_free_semaphores(list(self.sems.allocated().values()))
    self.nc.all_engine_barrier()


tile.TileContext._drain_and_barrier = _drain_and_barrier_split


def _split_waits(nc, cap=1):
    """Hoist excess sync-waits onto standalone EventSemaphore instructions.

    The pinned walrus rejects instructions carrying more than one sync-wait
    command ("Too many sync wait commands"), and rejects ANY sync-wait on the
    direct2d xbar-transpose DMA. Keep at most `cap` waits on each instruction
    (0 for InstDmaTransposeAnt) and emit the rest as dedicated same-engine
    wait instructions immediately before it.
    """
    n = 0
    for f in nc.m.functions:
        for blk in f.blocks:
            out = []
            for inst in blk.instructions:
                icap = 0 if isinstance(inst, mybir.InstDmaTransposeAnt) else cap
                si = inst.sync_info
                waits = list(si.on_wait) if si is not None and si.on_wait else []
                if len(waits) > icap:
                    keep = waits[-icap:] if icap else []
                    for wv in waits[: len(waits) - icap]:
                        n += 1
                        ev = mybir.InstEventSemaphore(
                            name=f"wsplit-{n}",
                            opcode="EventSemaphore",
                            engine=inst.engine,
                            debug=inst.debug,
                            ins=[],
                            outs=[],
                            descendants=None,
                            sync_info=mybir.SyncInfo(on_wait=[wv], on_update=[]),
                            bass_sim_breakpoint=False,
                            bass_priority=None,
                            bass_wait_until_ts=None,
                            bass_scheduled_tick=None,
                            bass_scheduled_proc=None,
                            bass_scheduled_scope=None,
                            bass_addl_debug=None,
                            bass_nofuse=True,
                        )
                        out.append(ev)
                    inst.sync_info = mybir.SyncInfo(
                        on_wait=keep, on_update=list(si.on_update)
                    )
                out.append(inst)
            blk.instructions[:] = out
    return n

# ---------------------------------------------------------------------------
# Shapes (hardcoded per spec)
# ---------------------------------------------------------------------------
B, L, D = 4, 4096, 1024
H, W = 16, 128
DH = D // H  # 64
N_CORES = 8
T = (B * L) // N_CORES  # 2048 tokens per core
NW = T // W  # 16 windows per core
KT = D // 128  # 8 k-tiles
NQK = 2 * D // 128  # 16 feature tiles of q,k
TC = 512  # tokens per window-block
WB = T // TC  # 4 window blocks
HP = H // 2  # 8 head pairs
SCALE = DH**-0.5  # 0.125
WS = 32.0  # fp8 weight scale
CAUSAL_SKIP = True
N_WARM = 32

F32 = mybir.dt.float32
BF16 = mybir.dt.bfloat16
F8 = mybir.dt.float8e4
DR = mybir.MatmulPerfMode.DoubleRow


def build_nc(split_waits=True):
    nc = bass.Bass()

    # xt*[p, kt, t] = x[token t, feature kt*128+p] as fp8 high+residual
    xth_in = nc.declare_dram_parameter("xth", [128, KT, T], F8, isOutput=False)
    xtl_in = nc.declare_dram_parameter("xtl", [128, KT, T], F8, isOutput=False)
    # w3*[p, kt, f] ~= 32*qkv_w[f, kt*128+p], f in [0,2048)  (q,k rows)
    w3h_in = nc.declare_dram_parameter("w3h", [128, KT, 2 * D], F8, isOutput=False)
    # wv*[p, kt, f] ~= 32*qkv_w[2D+f, kt*128+p]  (v rows)
    wvh_in = nc.declare_dram_parameter("wvh", [128, KT, D], F8, isOutput=False)
    wvl_in = nc.declare_dram_parameter("wvl", [128, KT, D], F8, isOutput=False)
    # wo*[p, kt, f] ~= 32*out_w[f, kt*128+p]
    woh_in = nc.declare_dram_parameter("woh", [128, KT, D], F8, isOutput=False)
    wol_in = nc.declare_dram_parameter("wol", [128, KT, D], F8, isOutput=False)
    # b3qk[2048]; q part pre-scaled by SCALE on host
    b3_in = nc.declare_dram_parameter("b3", [2 * D], F32, isOutput=False)
    # bo2 = out_b + out_w @ v_bias  (V bias folded through softmax)
    bo_in = nc.declare_dram_parameter("bo", [D], F32, isOutput=False)
    # y stored transposed: y_out[fb, p, t] = y[token t, feature fb*128+p]
    y_out = nc.declare_dram_parameter("y", [KT, 128, T], BF16, isOutput=True)

    with tile.TileContext(nc) as tc:
        with (
            tc.tile_pool(name="consts", bufs=1) as consts,
            tc.tile_pool(name="weights", bufs=1) as wpool,
            tc.tile_pool(name="xt_res", bufs=1) as xt_pool,
            tc.tile_pool(name="qkc", bufs=2) as qkc_pool,
            tc.tile_pool(name="vn", bufs=2) as vn_pool,
            tc.tile_pool(name="aot", bufs=2) as aot_pool,
            tc.tile_pool(name="aotb", bufs=2) as aotb_pool,
            tc.tile_pool(name="attn_sb", bufs=4) as attn_sb,
            tc.tile_pool(name="pt2", bufs=3) as pt2_pool,
            tc.tile_pool(name="y_sb", bufs=6) as y_sb_pool,
            tc.tile_pool(name="qk_ps", bufs=2, space="PSUM") as qk_ps,
            tc.tile_pool(name="sc_ps", bufs=3, space="PSUM") as sc_ps,
            tc.tile_pool(name="ao_ps", bufs=1, space="PSUM") as ao_ps,
            tc.tile_pool(name="y_ps", bufs=2, space="PSUM") as y_ps,
        ):
            # identity for PE transposes — built first so the PE warm-up can
            # start before the weight DMAs land
            from concourse.masks import make_identity

            id_bf16 = consts.tile([128, 128], BF16)
            make_identity(nc, id_bf16)

            # one-time causal mask (1 where k<=q, else 0), replicated over
            # the 4 windows of a block: applied to P with a single 4x-mode
            # DVE multiply per sub-head (the v5 affine_select equivalent,
            # ~4x cheaper and off the Pool engine)
            mask4 = consts.tile([128, 512], BF16)
            nc.vector.memset(mask4, 1.0)
            nc.gpsimd.affine_select(
                out=mask4.rearrange("p (i k) -> p i k", i=4),
                in_=mask4.rearrange("p (i k) -> p i k", i=4),
                compare_op=mybir.AluOpType.is_ge,
                fill=0.0,
                base=0,
                pattern=[[0, 4], [-1, W]],
                channel_multiplier=1,
            )

            def pad(n):
                """n dummy 128-col transposes (~53ns each at full clock)
                into a fresh sc-pool generation: keeps the PE busy through
                thin-filler stretches so the p-state ramp never resets (a
                single idle gap drops the PE to 0.65GHz for ~3us). Only used
                at warm-up and in the tail, where rotating the sc pool an
                extra step cannot stall future scores."""
                if n <= 0:
                    return
                scratch = sc_ps.tile([128, 512], BF16, name="sc")
                for i in range(n):
                    nc.tensor.transpose(
                        scratch[:, (i % 4) * W : (i % 4 + 1) * W],
                        id_bf16,
                        id_bf16,
                    )

            # --- DMA plan. All SWDGE (gpsimd/ACT) traffic serializes on one
            # ~344B/ns resource; the SP HWDGE queue is separate (~205B/ns).
            # HWDGE carries xth/xtl (chunk-major) + y stores; SWDGE carries
            # weights, sliced and ordered just ahead of consumption.
            # Whole-tensor weight DMAs: per-partition contiguous -> 1
            # descriptor/partition, one ~1us SWDGE submit each (submits run
            # on the Pool engine, so keeping their count low matters).
            w3h_sb = wpool.tile([128, KT, 2 * D], F8, name="w3h")
            xth_sb = xt_pool.tile([128, KT, T], F8, name="xth")
            xtl_sb = xt_pool.tile([128, KT, T], F8, name="xtl")

            # V weights ride the fast HWDGE queue interleaved with the
            # first x chunks: the prologue leads with V-projection groups
            # while the (larger) w3 stream lands behind on SWDGE
            wvh_sb = wpool.tile([128, KT, D], F8, name="wvh")
            wvl_sb = wpool.tile([128, KT, D], F8, name="wvl")

            def xt_chunk(cb, quarters=1):
                c0 = cb * TC
                q = TC // quarters
                for k in range(quarters):
                    a, b = c0 + k * q, c0 + (k + 1) * q
                    for dst, src in ((xth_sb, xth_in), (xtl_sb, xtl_in)):
                        nc.sync.dma_start(out=dst[:, :, a:b], in_=src[:, :, a:b])

            # startup: window-sized first x piece + wv halves so the first
            # V-projection group can start early
            for dst, src in ((xth_sb, xth_in), (xtl_sb, xtl_in)):
                nc.sync.dma_start(out=dst[:, :, 0:W], in_=src[:, :, 0:W])
            nc.sync.dma_start(out=wvh_sb[:, :, 0:TC], in_=wvh_in[:, :, 0:TC])
            nc.sync.dma_start(out=wvl_sb[:, :, 0:TC], in_=wvl_in[:, :, 0:TC])
            for dst, src in ((xth_sb, xth_in), (xtl_sb, xtl_in)):
                nc.sync.dma_start(out=dst[:, :, W:TC], in_=src[:, :, W:TC])
            nc.sync.dma_start(out=wvh_sb[:, :, TC:], in_=wvh_in[:, :, TC:])
            nc.sync.dma_start(out=wvl_sb[:, :, TC:], in_=wvl_in[:, :, TC:])
            for cb in range(1, WB):
                xt_chunk(cb)

            b3_sb = consts.tile([128, NQK], F32)
            nc.gpsimd.dma_start(
                out=b3_sb, in_=b3_in[:].rearrange("(a p) -> p a", p=128)
            )
            for sl in range(4):
                f0, f1 = sl * TC, (sl + 1) * TC
                nc.gpsimd.dma_start(
                    out=w3h_sb[:, :, f0:f1], in_=w3h_in[:, :, f0:f1]
                )
            woh_sb = wpool.tile([128, KT, D], F8, name="woh")
            wol_sb = wpool.tile([128, KT, D], F8, name="wol")

            # bo_sb[p, fb] = bo[fb*128 + p]: per-partition bias slices for
            # the transposed out-proj epilogue
            bo_sb = consts.tile([128, KT], F32)

            def dr3(ps, stat_hl, mov_hl, start_grp=True, lo_mov_first=False,
                    n_passes=3):
                """Residual-compensated DoubleRow accumulation (2 or 3
                passes): stat/mov are (high, low) slice-getter pairs; slices
                take the k-tile-pair index j and return [128, 2, *] APs."""
                sh, sl = stat_hl
                mh, ml = mov_hl
                if lo_mov_first:
                    passes = [(sh, mh), (sh, ml), (sl, mh)]
                else:
                    passes = [(sh, mh), (sl, mh), (sh, ml)]
                passes = passes[:n_passes]
                np_ = len(passes)
                for pi, (sg, mg) in enumerate(passes):
                    for j in range(KT // 2):
                        nc.tensor.matmul(
                            ps,
                            sg(j),
                            mg(j),
                            start=(pi == 0 and j == 0 and start_grp),
                            stop=(pi == np_ - 1 and j == KT // 2 - 1),
                            perf_mode=DR,
                        )

            # ------------------------------------------------------------------
            # Emission groups: each is ~12 chained DoubleRow matmuls plus an
            # ACT/DVE epilogue; groups are the filler units interleaved into
            # the attention blocks.
            # ------------------------------------------------------------------

            def proj_block(wb, interleave=True):
                c0 = wb * TC
                qk = [
                    qkc_pool.tile([128, TC], BF16, name=f"qk{ft}")
                    for ft in range(NQK)
                ]
                vn = [
                    vn_pool.tile([128, D], BF16, name=f"vn{i}") for i in range(4)
                ]
                groups = []

                def g_qk(ft):
                    def g():
                        ps = qk_ps.tile([128, TC], F32, name="ps_qk")
                        fsl = slice(ft * 128, (ft + 1) * 128)
                        dr3(
                            ps,
                            (
                                lambda j: w3h_sb[:, 2 * j : 2 * j + 2, fsl],
                                None,
                            ),
                            (
                                lambda j: xth_sb[:, 2 * j : 2 * j + 2, c0 : c0 + TC],
                                lambda j: xtl_sb[:, 2 * j : 2 * j + 2, c0 : c0 + TC],
                            ),
                            lo_mov_first=True,
                            n_passes=2,
                        )
                        sc = (SCALE if ft < KT else 1.0) / WS
                        # alternate the PSUM->SBUF epilogue between ACT and
                        # DVE so neither in-order queue backs up ahead of the
                        # chain-critical exp / softmax ops; on seam calls
                        # everything goes to DVE so exp never queues behind
                        if ft % 2 == 0:
                            nc.scalar.activation(
                                out=qk[ft],
                                in_=ps,
                                func=mybir.ActivationFunctionType.Identity,
                                bias=b3_sb[:, ft : ft + 1],
                                scale=sc,
                            )
                        else:
                            nc.vector.tensor_scalar(
                                out=qk[ft],
                                in0=ps,
                                scalar1=sc,
                                scalar2=b3_sb[:, ft : ft + 1],
                                op0=mybir.AluOpType.mult,
                                op1=mybir.AluOpType.add,
                            )

                    return g

                def g_v(i, fo):
                    def g():
                        f0 = fo * TC
                        t0 = c0 + i * W
                        fsl = slice(f0, f0 + TC)
                        ps = qk_ps.tile([128, TC], F32, name="ps_qk")
                        dr3(
                            ps,
                            (
                                lambda j: xth_sb[:, 2 * j : 2 * j + 2, t0 : t0 + W],
                                lambda j: xtl_sb[:, 2 * j : 2 * j + 2, t0 : t0 + W],
                            ),
                            (
                                lambda j: wvh_sb[:, 2 * j : 2 * j + 2, fsl],
                                lambda j: wvl_sb[:, 2 * j : 2 * j + 2, fsl],
                            ),
                        )
                        nc.scalar.activation(
                            out=vn[i][:, f0 : f0 + TC],
                            in_=ps,
                            func=mybir.ActivationFunctionType.Identity,
                            scale=1.0 / WS,
                        )

                    return g

                if interleave:
                    # q/k interleaved so a prefix covers whole heads (for the
                    # just-in-time carry into wb3)
                    for h in range(KT):
                        groups.append(g_qk(h))
                        groups.append(g_qk(KT + h))
                    for i in range(4):
                        for fo in range(2):
                            groups.append(g_v(i, fo))
                else:
                    # prologue: V first (wv lands first), fo-major so the
                    # first wv half feeds four groups, then sequential ft
                    # matching the w3 f-slice DMA arrival order
                    for fo in range(2):
                        for i in range(4):
                            groups.append(g_v(i, fo))
                    for ft in range(NQK):
                        groups.append(g_qk(ft))
                return qk, vn, groups

            def outp_block(wb, aoth, aotl):
                """Out-projection, transposed: stationary = weights, moving =
                aoT, so the output is y^T [feat-part, token-cols]. The bias
                is then per-partition and folds into the ACT epilogue for
                free; the host un-transposes y."""
                c0 = wb * TC
                groups = []

                def g_out(fb):
                    def g():
                        fsl = slice(fb * 128, (fb + 1) * 128)
                        yp = y_ps.tile([128, TC], F32, name="yp")
                        dr3(
                            yp,
                            (
                                lambda j: woh_sb[:, 2 * j : 2 * j + 2, fsl],
                                lambda j: wol_sb[:, 2 * j : 2 * j + 2, fsl],
                            ),
                            (
                                lambda j: aoth[j][:, :, :],
                                lambda j: aotl[j][:, :, :],
                            ),
                        )
                        yt = y_sb_pool.tile([128, TC], BF16, name="yt")
                        nc.scalar.activation(
                            out=yt,
                            in_=yp,
                            func=mybir.ActivationFunctionType.Identity,
                            bias=bo_sb[:, fb : fb + 1],
                            scale=1.0 / WS,
                        )

                        # defer the store's EMISSION one call: the in-order
                        # SP sequencer parks on a DMA whose data isn't ready,
                        # which would block every later P^T xbar dispatch
                        def store():
                            nc.sync.dma_start(
                                out=y_out[fb, :, c0 : c0 + TC],
                                in_=yt,
                            )

                        store_q.append(store)

                    return g

                for fb in range(KT):
                    groups.append(g_out(fb))
                return groups

            def sc_alloc():
                """Allocate the next call's score PSUM tiles and set the
                fully-masked [q<64, k>=64] window quarters to -1e30 (exp
                then writes exact zeros there). Done a call AHEAD so the
                memsets never gate the next call's score matmuls."""
                sc = [
                    sc_ps.tile([128, 512], F32, name="sc") for s in range(2)
                ]
                for s in range(2):
                    nc.vector.memset(
                        sc[s][0:DH, :].rearrange("p (i k) -> p i k", i=4)[
                            :, :, DH:
                        ],
                        -1.0e30,
                    )
                return sc

            def attention(wb, hp, qk, vn, aot_bf, aoth, aotl, filler, deferred,
                          av_pending, sc_q, pad_n=0, inline=False):
                """Attention for head-pair hp of block wb.

                Emits scores + softmax + the P^T xbar-DMA for this pair, then
                runs the attn@V stage of the pair issued TWO calls ago (so the
                ~3us DMA-transpose latency hides behind two call periods),
                then deferred fp8 splits and the filler groups."""
                sc = sc_q.pop(0)
                p_all = attn_sb.tile([128, 1024], BF16, name="p_all")
                for i in range(4):
                    cl = i * W
                    for s in range(2):
                        r0 = s * DH
                        # left half: all q rows vs k in [0,64)
                        nc.tensor.matmul(
                            sc[s][:, cl : cl + DH],
                            qk[hp][r0 : r0 + DH, cl : cl + W],
                            qk[KT + hp][r0 : r0 + DH, cl : cl + DH],
                            start=True,
                            stop=True,
                        )
                        # bottom-right: q in [64,128) vs k in [64,128)
                        nc.tensor.matmul(
                            sc[s][DH:, cl + DH : cl + W],
                            qk[hp][r0 : r0 + DH, cl + DH : cl + W],
                            qk[KT + hp][r0 : r0 + DH, cl + DH : cl + W],
                            start=True,
                            stop=True,
                        )
                # P stays UNNORMALIZED through the transpose and attn@V;
                # normalization happens on the attn@V output where per-q is
                # per-partition. The critical chain is just scores->exp->DMA;
                # rowsums/recip run concurrently with the xbar transpose.
                sums = attn_sb.tile([128, 8], F32, name="sums")
                recip = attn_sb.tile([128, 8], F32, name="recip")
                pt2 = None if inline else pt2_pool.tile(
                    [128, 8, 128], BF16, name="pt2"
                )
                # next call's score banks: allocated here (before the exps)
                # so their DVE memsets fill the time DVE would otherwise
                # spend parked waiting for exp before the mask multiplies
                sc_q.append(sc_alloc())
                for s in range(2):
                    psl = p_all[:, s * 512 : (s + 1) * 512]
                    nc.scalar.activation(
                        out=psl,
                        in_=sc[s],
                        func=mybir.ActivationFunctionType.Exp,
                    )
                    # causal mask: zero every k>q entry (the quarter was
                    # pre-set to -1e30 in PSUM so its exp is finite)
                    nc.vector.tensor_mul(out=psl, in0=psl, in1=mask4)
                    # pt2[k, 4s+i, q] = E[q, i*128+k], straight off the mask
                    if not inline:
                        nc.sync.dma_start_transpose(
                            out=pt2[:, 4 * s : 4 * s + 4, :],
                            in_=psl,
                        )

                # rowsums + recip on DVE, off the critical chain (they are
                # only needed by the attn@V stage two calls later)
                for s in range(2):
                    nc.vector.reduce_sum(
                        out=sums[:, 4 * s : 4 * s + 4],
                        in_=p_all[:, s * 512 : (s + 1) * 512].rearrange(
                            "p (i k) -> p i k", i=4
                        ),
                        axis=mybir.AxisListType.X,
                    )
                nc.vector.reciprocal(out=recip, in_=sums)
                if inline:
                    # normalize P in place; the inline (v5-style) tail path
                    # transposes on the PE and needs normalized P
                    for s in range(2):
                        for i in range(4):
                            sl = p_all[
                                :, s * 512 + i * W : s * 512 + (i + 1) * W
                            ]
                            nc.vector.tensor_scalar_mul(
                                out=sl,
                                in0=sl,
                                scalar1=recip[:, 4 * s + i : 4 * s + i + 1],
                            )

                def av_stage():
                    # aoU[q, i*128 + s*64 + d] = sum_k E[q,k] v[k,d]
                    # (64-wide moving => half the PE cost of the [dh, q] form)
                    ao = ao_ps.tile([128, 512], F32, name="ao")
                    for i in range(4):
                        for s in range(2):
                            f0 = hp * 128 + s * DH
                            c0 = i * W + s * DH
                            nc.tensor.matmul(
                                ao[:, c0 : c0 + DH],
                                pt2[:, 4 * s + i, :],
                                vn[i][:, f0 : f0 + DH],
                                start=True,
                                stop=True,
                            )
                    ao_bf = attn_sb.tile([128, 512], BF16, name="ao_bf")
                    nc.scalar.activation(
                        out=ao_bf, in_=ao,
                        func=mybir.ActivationFunctionType.Identity,
                    )
                    # softmax normalization, deferred to here: per-q scale
                    # is per-partition in this layout; on Pool, keeping DVE
                    # free for the mask/sums chain
                    for i in range(4):
                        for s in range(2):
                            c0 = i * W + s * DH
                            nc.gpsimd.tensor_scalar_mul(
                                out=ao_bf[:, c0 : c0 + DH],
                                in0=ao_bf[:, c0 : c0 + DH],
                                scalar1=recip[:, 4 * s + i : 4 * s + i + 1],
                            )
                    # aoT via xbar: aot_bf[s*64+d, hp, i*128+q] = ao_bf[q, ...]
                    # EMITTED one call later (via store_q) so the in-order SP
                    # sequencer never parks on the norm-mul semaphores and
                    # blocks subsequent P^T dispatches
                    def ao_dma():
                        nc.sync.dma_start_transpose(
                            out=aot_bf[:, hp, :].rearrange(
                                "p (i q) -> p i q", i=4
                            ),
                            in_=ao_bf,
                        )
                        deferred.append(aot_split)

                    store_q.append(ao_dma)

                    def aot_split():
                        # split halves on Pool in steady state (it is
                        # otherwise idle); the last calls' splits run in the
                        # drained tail where DVE is free — put those there
                        eng = nc.vector if hp >= HP - 2 and wb == WB - 1 else nc.gpsimd
                        eng.tensor_copy(
                            out=aoth[hp // 2][:, hp % 2, :],
                            in_=aot_bf[:, hp, :],
                        )
                        eng.tensor_sub(
                            out=aotl[hp // 2][:, hp % 2, :],
                            in0=aot_bf[:, hp, :],
                            in1=aoth[hp // 2][:, hp % 2, :],
                        )

                # fp8 splits appended by earlier av stages: consume the ones
                # queued BEFORE this call so each split trails its xbar DMA by
                # a full call period (emitting it sooner parks the in-order
                # DVE queue on the DMA semaphore).
                splits_now = list(deferred)
                deferred.clear()

                pad(pad_n)
                if not inline:
                    av_pending.append(av_stage)
                    # uniform depth-2 deferral: the shortened chain (scores
                    # -> exp -> xbar DMA, ~4us) fits two call periods
                    if len(av_pending) > 2:
                        av_pending.pop(0)()
                elif av_pending:
                    # inline calls still drain one pending stage so nothing
                    # piles up for the tail
                    av_pending.pop(0)()
                for d in splits_now:
                    d()
                stores_now = list(store_hold)
                store_hold[:] = store_q
                store_q.clear()
                for st in stores_now:
                    st()
                for g in filler:
                    g()
                if inline:
                    # v5-style finish for the block's LAST head-pair: PE
                    # transposes + direct aoT attn@V + SBUF split, no xbar
                    # DMAs — so no attention chain ever crosses a block seam
                    pts = []
                    for s in range(2):
                        ptp = ao_ps.tile([128, 512], BF16, name="ao")
                        for i in range(4):
                            nc.tensor.transpose(
                                ptp[:, i * W : (i + 1) * W],
                                p_all[:, s * 512 + i * W : s * 512 + (i + 1) * W],
                                id_bf16,
                            )
                        ptsb = attn_sb.tile([128, 512], BF16, name=f"pts{s}")
                        if s == 0:
                            nc.scalar.activation(
                                out=ptsb, in_=ptp,
                                func=mybir.ActivationFunctionType.Identity,
                            )
                        else:
                            nc.vector.tensor_copy(out=ptsb, in_=ptp)
                        pts.append(ptsb)
                    ao = ao_ps.tile([128, 512], F32, name="ao")
                    for i in range(4):
                        for s in range(2):
                            f0 = hp * 128 + s * DH
                            nc.tensor.matmul(
                                ao[s * DH : (s + 1) * DH, i * W : (i + 1) * W],
                                vn[i][:, f0 : f0 + DH],
                                pts[s][:, i * W : (i + 1) * W],
                                start=True,
                                stop=True,
                                tile_position=(0, s * DH),
                            )
                    ao_sb = attn_sb.tile([128, 512], F32, name="ao_sb")
                    nc.scalar.activation(
                        out=ao_sb, in_=ao,
                        func=mybir.ActivationFunctionType.Identity,
                    )

                    def inline_split():
                        nc.vector.tensor_copy(
                            out=aoth[hp // 2][:, hp % 2, :], in_=ao_sb
                        )
                        nc.vector.tensor_sub(
                            out=aotl[hp // 2][:, hp % 2, :],
                            in0=ao_sb,
                            in1=aoth[hp // 2][:, hp % 2, :],
                        )

                    deferred.append(inline_split)

            # ------------------------------------------------------------------
            # Pipeline: prologue proj(0); per wb: attention(wb) interleaved
            # with proj(wb+1) and outp(wb-1); epilogue outp(3).
            # ------------------------------------------------------------------
            # PE warm-up: dummy transposes while the first weights stream
            # in — keeps the PE p-state ramp running and costs nothing (the
            # y_ps banks see real use only from wb1)
            pad(N_WARM)

            qk_cur, vn_cur, groups0 = proj_block(0, interleave=False)
            for g in groups0:
                g()

            # out-proj weights + bias: needed only from wb2 on; their SWDGE
            # submits are spread across wb0's head-pairs (emitted in the
            # pipeline loop) so they neither displace the w3 stream nor
            # stall the Pool queue in one lump
            def late_weight_dmas():
                yield lambda: nc.gpsimd.dma_start(out=woh_sb, in_=woh_in[:])
                yield lambda: nc.gpsimd.dma_start(out=wol_sb, in_=wol_in[:])
                yield lambda: nc.gpsimd.dma_start(
                    out=bo_sb, in_=bo_in[:].rearrange("(a p) -> p a", p=128)
                )

            late_dmas = late_weight_dmas()

            aos = {}
            carry = []  # proj(3) groups deferred from wb2 into wb3
            deferred = []  # fp8 aoT splits, consumed one call after queueing
            av_pending = []  # attn@V stages, run two calls after their scores
            store_q = []  # y stores, emitted one call after their epilogue
            store_hold = []  # extra call of deferral for DMA emissions
            sc_q = [sc_alloc()]  # score banks, masked a call ahead
            for wb in range(WB):
                filler = list(carry)
                carry = []
                if wb + 1 < WB:
                    qk_nxt, vn_nxt, pgroups = proj_block(wb + 1)
                    if wb + 1 == WB - 1:
                        # last proj block: emit head-pairs 0..3 + all of V
                        # now, defer pairs 4..7 into wb3 (ahead of their use)
                        filler.extend(pgroups[:8] + pgroups[NQK:])
                        carry = pgroups[8:NQK]
                    else:
                        filler.extend(pgroups)
                # out-projection of the previous block, spread over calls
                # 3..7 of this block (its fp8 splits land by call 2 thanks
                # to the seam flush) and interleaved with the proj groups so
                # the yt/y-store load never bursts; outp(3) runs in the tail
                if wb - 1 in aos:
                    og = outp_block(wb - 1, *aos[wb - 1])
                    n_pre = 4 * (len(filler) + len(og)) // 8
                    pre, post = filler[:n_pre], filler[n_pre:]
                    mixed = []
                    na, nb = len(post), len(og)
                    ia = ib = 0
                    for k in range(na + nb):
                        if ia * nb <= ib * na and ia < na:
                            mixed.append(post[ia])
                            ia += 1
                        elif ib < nb:
                            mixed.append(og[ib])
                            ib += 1
                        else:
                            mixed.append(post[ia])
                            ia += 1
                    filler = pre + mixed

                aot_bf = aotb_pool.tile([128, KT, 4 * W], BF16, name="aot_bf")
                # per-head-pair fp8 tiles: out-proj matmul j then waits only
                # pair j's split instead of the whole block's last split
                aoth = [
                    aot_pool.tile([128, 2, 4 * W], F8, name=f"aoth{j}")
                    for j in range(KT // 2)
                ]
                aotl = [
                    aot_pool.tile([128, 2, 4 * W], F8, name=f"aotl{j}")
                    for j in range(KT // 2)
                ]
                n = len(filler)
                for hp in range(HP):
                    if wb == 0 and hp in (2, 4, 6):
                        next(late_dmas, lambda: None)()
                    lo = n * hp // HP
                    hi = n * (hp + 1) // HP
                    attention(
                        wb, hp, qk_cur, vn_cur, aot_bf, aoth, aotl,
                        filler[lo:hi], deferred, av_pending, sc_q,
                        inline=(hp >= HP - 3),
                    )

                aos[wb] = (aoth, aotl)
                if wb + 1 < WB:
                    qk_cur, vn_cur = qk_nxt, vn_nxt

            # Tail: flush the last four attn@V stages and fp8 splits with
            # padding so the PE clock stays up while the last xbar DMAs
            # land; outp(3) last (it needs the final splits).
            for d in deferred:
                d()
            deferred.clear()
            while av_pending:
                av_pending.pop(0)()
                pad(10)
            for st in store_hold + store_q:
                st()
            store_hold.clear()
            store_q.clear()
            pad(52)
            for d in deferred:
                d()
            deferred.clear()
            for g in outp_block(3, *aos[3]):
                g()
            for st in store_q:
                st()
            store_q.clear()

    if split_waits:
        _split_waits(nc)
    return nc


def prep_inputs(x, qkv_w, qkv_b, out_w, out_b):
    """Host-side prep: slice tokens per core, transpose weights, split
    everything into fp8 high + residual parts."""
    x = np.ascontiguousarray(np.asarray(x, dtype=np.float32).reshape(B * L, D))
    qkv_w = np.asarray(qkv_w, dtype=np.float32)
    qkv_b = np.asarray(qkv_b, dtype=np.float32)
    out_w = np.asarray(out_w, dtype=np.float32)
    out_b = np.asarray(out_b, dtype=np.float32)

    import ml_dtypes

    E4 = ml_dtypes.float8_e4m3

    def split8(a):
        hi = a.astype(E4)
        lo = (a - hi.astype(np.float32)).astype(E4)
        return np.ascontiguousarray(hi), np.ascontiguousarray(lo)

    # xt[p, kt, t] = x[core*T + t, kt*128 + p]
    xt_all = x.reshape(N_CORES, T, KT, 128).transpose(0, 3, 2, 1)
    xth, xtl = split8(xt_all)
    # w3[p, kt, f] = 32 * qkv_w[f, kt*128 + p]  (q,k rows)
    w3h, _ = split8(WS * qkv_w[: 2 * D].reshape(2 * D, KT, 128).transpose(2, 1, 0))
    # wv[p, kt, f] = 32 * qkv_w[2D + f, kt*128 + p]
    wvh, wvl = split8(WS * qkv_w[2 * D :].reshape(D, KT, 128).transpose(2, 1, 0))
    # wo[p, kt, f] = 32 * out_w[f, kt*128 + p]
    woh, wol = split8(WS * out_w.reshape(D, KT, 128).transpose(2, 1, 0))
    b3 = qkv_b[: 2 * D].copy()
    b3[:D] *= SCALE
    # V bias folded through softmax into the output bias
    bo2 = out_b + out_w @ qkv_b[2 * D :]

    in_maps = []
    for c in range(N_CORES):
        in_maps.append(
            {
                "xth": xth[c],
                "xtl": xtl[c],
                "w3h": w3h,
                "wvh": wvh,
                "wvl": wvl,
                "woh": woh,
                "wol": wol,
                "b3": b3,
                "bo": bo2,
            }
        )
    return in_maps


_NC_CACHE = None


def kernel(x, qkv_w, qkv_b, out_w, out_b):
    global _NC_CACHE
    if _NC_CACHE is None:
        _NC_CACHE = build_nc()
    nc = _NC_CACHE
    in_maps = prep_inputs(x, qkv_w, qkv_b, out_w, out_b)
    res = run_bass_kernel_spmd(nc, in_maps, core_ids=list(range(N_CORES)))
    y = np.concatenate(
        [
            # y comes back transposed: [KT, 128, T] = y^T [D, T]
            np.asarray(res.results[c]["y"], dtype=np.float32).reshape(D, T).T
            for c in range(N_CORES)
        ],
        axis=0,
    )
    return y.reshape(B, L, D)

